# revision 7
# baseline (speedup 1.0000x reference)
"""Covariance pooling kernel for Trainium2 (8 NeuronCores, data-parallel over batch).

y[b] = (1/M) * (x[b] - mean(x[b])) @ (x[b] - mean(x[b]))^T  with x[b] [C=128, M=4096].

HBM-read bound: 16.78 MB fp32 per core; the SWDGE cast stream sustains
~405 GB/s/core when descriptors are >=1KB-per-partition rows, so the whole
kernel is paced by ~41 us of streaming plus fixed preamble/epilogue:
  - SWDGE cast DMAs (fp32 HBM -> fp8 SBUF): first/last batch split 4x (1KB
    write rows keep line rate; finer splits measured at 143 GB/s), middles
    whole; all 8 fp8 batches stay resident so every DMA enqueues up front
  - the PE pipeline rate is set by LDWEIGHTS serialization (no FWL in this
    toolchain; walrus rejects plane-stride DoubleRow weights, so the byte
    interleave + SwInterleave gram is the cheapest legal form):
      * per chunk: one NORMAL-mode fp8 matmul against the identity
        (out = chunk^T, fp32 PSUM) -- normal-mode LDWEIGHTS measured ~87ns
        vs ~99ns for transpose-mode
      * per pair: one DoubleRowSwInterleave gram matmul (K=256, ~134ns LDW)
    => ~308ns of LDW per 512 spatial values, just under the stream rate
  - DVE/ACT alternate interleave pair-copies (PSUM fp32 -> SBUF fp8 pairs);
    a constant ones column feeds row sums through the gram matmul
  - ~14 identity warm-up matmuls flip the HAM clock gate (1.2 -> 2.4 GHz)
    before batch 0's data lands, so batch 0 runs warm
  - per-batch y writes (HWDGE, 516B rows) overlap the stream
  - DoubleRowSwInterleave reads stationary columns reversed, so PSUM ends as
    [P@G | P@s] (rows flipped); the host un-flips rows and applies the
    rank-1 mean correction (0.005% of the FLOPs) while gathering shards
"""

import numpy as np

import ml_dtypes
import concourse.bass as bass
import concourse.tile as tile
from concourse import bacc, mybir
from concourse.bass_utils import run_bass_kernel_spmd

N_CORES = 8
B_FULL = 64
B_CORE = B_FULL // N_CORES  # 8 batches per core
C = 128
M = 4096  # 64*64 spatial
PAIRS = M // 256  # 16 chunk pairs per batch
NSLOT = 8  # SBUF pair-slot ring
WARMUP = 14  # identity matmuls to flip HAM before real work
F32 = mybir.dt.float32
FP8 = mybir.dt.float8e4
COPY = mybir.ActivationFunctionType.Copy
DRSW = mybir.MatmulPerfMode.DoubleRowSwInterleave

_CACHE: dict = {}


def _build_program() -> bass.Bass:
    nc = bacc.Bacc()
    x = nc.declare_dram_parameter("x", [B_CORE, C, M], F32, isOutput=False)
    ident8 = nc.declare_dram_parameter("ident8", [C, C], FP8, isOutput=False)
    y = nc.declare_dram_parameter("y", [B_CORE, C, 129], F32, isOutput=True)

    with tile.TileContext(nc) as tc:
        with (
            tc.tile_pool(name="singles", bufs=1) as singles,
            tc.tile_pool(name="yout", bufs=3) as yout_pool,
            tc.tile_pool(name="tp", bufs=5, space="PSUM") as tp_pool,
            tc.tile_pool(name="gram", bufs=3, space="PSUM") as gram_pool,
        ):
            identity8 = singles.tile([C, C], FP8)
            nc.sync.dma_start(identity8, ident8[:, :])

            # whole input, fp8, resident: DMAs enqueue back-to-back with no
            # reuse hazards; 1-4KB-per-row descriptors keep line rate
            xb = singles.tile([C, B_CORE, M], FP8)
            splits = {0: 4, B_CORE - 1: 4}  # early PE start / short tail
            for b in range(B_CORE):
                n = splits.get(b, 1)
                step = M // n
                for h in range(n):
                    nc.gpsimd.dma_start(
                        xb[:, b, h * step : (h + 1) * step],
                        x[b][:, h * step : (h + 1) * step],
                    )

            # pair slots: fp8 byte 2c+t = chunk t col c; col 128 = ones
            # (feeds row sums through the gram matmul), col 129 = zero pad
            xt = singles.tile([C, NSLOT, 130, 2], FP8)
            nc.vector.memset(xt[:, :, 128, :], 1.0)
            nc.vector.memset(xt[:, :, 129, :], 0.0)

            # HAM warm-up: ~3.4us of PE activity flips the clock gate to
            # 2.4 GHz before batch 0's data arrives.  Gated on a memset junk
            # tile (available right after the preamble) rather than the
            # identity DMA, so the dummies start ~2.5us earlier.
            junk = singles.tile([C, C], FP8)
            nc.vector.memset(junk, 1.0)
            for w in range(WARMUP):
                warm = tp_pool.tile([C, 2, 128], F32, tag="tp")
                nc.tensor.matmul(warm[:, 0, :], junk, junk)

            for b in range(B_CORE):
                gram = gram_pool.tile([C, 130], F32)
                for p in range(PAIRS):
                    tp = tp_pool.tile([C, 2, 128], F32, tag="tp")
                    for t in range(2):
                        k = 2 * p + t
                        # normal-mode transpose: chunk^T = lhsT.T @ I
                        nc.tensor.matmul(
                            tp[:, t, :],
                            xb[:, b, k * 128 : (k + 1) * 128],
                            identity8,
                        )
                    s = p % NSLOT
                    # interleave for DRSW: dst byte (c, t) <- tp[t, c]
                    dst = xt[:, s, 0:128, :]
                    src = tp.rearrange("p t c -> p c t")
                    if p % 2 == 0:
                        nc.vector.tensor_copy(dst, src)
                    else:
                        nc.scalar.activation(dst, src, COPY)
                    nc.tensor.matmul(
                        gram,
                        xt[:, s, 0:128, :],
                        xt[:, s, 0:130, :].rearrange("p c t -> p t c"),
                        start=(p == 0),
                        stop=(p == PAIRS - 1),
                        perf_mode=DRSW,
                    )

                y_tile = yout_pool.tile([C, 129], F32)
                nc.vector.tensor_scalar_mul(y_tile, gram[:, 0:129], 1.0 / M)
                nc.sync.dma_start(y[b], y_tile)

    nc.compile()
    return nc


def _get_program() -> bass.Bass:
    if "nc" not in _CACHE:
        _CACHE["nc"] = _build_program()
    return _CACHE["nc"]


def _run(x: np.ndarray, **spmd_kwargs):
    x = np.ascontiguousarray(np.asarray(x), dtype=np.float32)
    assert x.shape == (B_FULL, C, 64, 64), x.shape
    xf = x.reshape(B_FULL, C, M)
    shards = np.split(xf, N_CORES, axis=0)
    ident8 = np.eye(C, dtype=ml_dtypes.float8_e4m3)
    in_maps = [{"x": s, "ident8": ident8} for s in shards]
    nc = _get_program()
    res = run_bass_kernel_spmd(nc, in_maps, list(range(N_CORES)), **spmd_kwargs)
    raw = np.concatenate([res.results[i]["y"] for i in range(N_CORES)], axis=0)
    # raw[b] = [P@G | P@s] / M (rows flipped by DoubleRowSwInterleave).
    # Un-flip and apply the rank-1 mean correction: y = G/M - (s/M)(s/M)^T
    g_flip = raw[:, ::-1, 0:128]
    sv = raw[:, ::-1, 128]  # s[c]/M, straight channel order
    out = g_flip - sv[:, :, None] * sv[:, None, :]
    return np.ascontiguousarray(out, dtype=np.float32), res


def kernel(x: np.ndarray) -> np.ndarray:
    out, _ = _run(x)
    return out
